# revision 26
# baseline (speedup 1.0000x reference)
"""CrissCrossAttention Trainium2 kernel (v2).

Sharding: 8 cores = 4 samples x 2 row-halves. Each core's sample is rolled so
its 49 rows sit at local rows [0, 49) (column attention is permutation-
invariant over the column index, so rolling is safe and keeps the SPMD
program identical across cores).

Math (per core, local rows j in [0,49), cols x in [0,97)):
  q|k = Wqk @ x + bqk                  (fp16 matmuls, PSUM fp32)
  E_row[(j,x), i] = q(j,x).k(j,i)      E_col[(j,x), i] = q(j,x).k(i,x)
  col diag (i==j) masked to -300 in PSUM pre-exp; P = exp(E) unnormalized
  (no max-shift; |E| <~ 45 fits fp32); S accumulated by the exp's accum_out.
  U = P_row @ X_row + P_col @ X_col    (bf16, channel-major in z)
  out = (Wv @ U + bv x S) * (gamma/S) + x     (rank-1 bias matmul makes the
  final stage a pure scale-and-add; exact by linearity)

DMA strategy: all bulk transfers ride SWDGE (nc.gpsimd) which sprays
descriptors across all 16 SDMA engines; host-side layouts are chosen so
every transfer has multi-KB contiguous per-partition lines.  The k mirror
(partition shift) uses the scalar HWDGE ring to stay off the sync ring.
"""

import os
import numpy as np
import ml_dtypes

import concourse.bacc as bacc
import concourse.bass as bass
import concourse.tile as tile
import concourse.mybir as mybir
from concourse.bass_utils import run_bass_kernel_spmd
from concourse.masks import make_identity

F32 = mybir.dt.float32
F16 = mybir.dt.float16
BF16 = mybir.dt.bfloat16
AF = mybir.ActivationFunctionType
AX = mybir.AxisListType
ALU = mybir.AluOpType

C = 512
CC = 4          # 4 chunks of 128 channels
CQ = 64
H = W = 97
NPIX = H * W    # 9409
R = 49          # rows per core (halves overlap at row 48)
PX = R * W      # 4753
GP = 4          # softmax/agg group
GDC = 12        # xtc chunk (x's per dma)
GDR = 8         # xtr chunk (j's per dma)
SLAB = 512      # projection slab (pixels per dma)
T4 = 512        # phase-4 pixel tile

_cache = {}
last_results = None


def _groups(total, g):
    out = []
    i = 0
    while i < total:
        out.append((i, min(g, total - i)))
        i += g
    return out


def _build(gamma: float):
    nc = bacc.Bacc("TRN2", target_bir_lowering=False, debug=False,
                   enable_asserts=False)

    xcd = nc.dram_tensor("xcd", [128, NPIX, CC], F16, kind="ExternalInput")
    xtcd = nc.dram_tensor("xtcd", [128, W, C], BF16, kind="ExternalInput")
    xtrd = nc.dram_tensor("xtrd", [128, R, C], BF16, kind="ExternalInput")
    xresd = nc.dram_tensor("xresd", [128, CC, PX], F16, kind="ExternalInput")
    wqk = nc.dram_tensor("wqk", [128, CC, 128], F16, kind="ExternalInput")
    bias2 = nc.dram_tensor("bias2", [128, 128], F16, kind="ExternalInput")
    ones2 = nc.dram_tensor("ones2", [128, 512], F16, kind="ExternalInput")
    wv4d = nc.dram_tensor("wv4d", [128, CC, CC, 128], BF16, kind="ExternalInput")
    gbvd = nc.dram_tensor("gbvd", [128, CC], F32, kind="ExternalInput")
    out = nc.dram_tensor("out", [128, CC, PX], F32, kind="ExternalOutput")
    dbg = None

    with tile.TileContext(nc) as tc:
        with (
            tc.tile_pool(name="singles", bufs=1) as singles,
            tc.tile_pool(name="xcp", bufs=2) as xcp,
            tc.tile_pool(name="kstp", bufs=2) as kstp,
            tc.tile_pool(name="xtcp", bufs=2) as xtcp,
            tc.tile_pool(name="xtrp", bufs=2) as xtrp,
            tc.tile_pool(name="xresp", bufs=2) as xresp,
            tc.tile_pool(name="ptp", bufs=2) as ptp,
            tc.tile_pool(name="outp", bufs=2) as outp,
            tc.tile_pool(name="sbtp", bufs=2) as sbtp,
            tc.tile_pool(name="ps_e2", bufs=4, space="PSUM") as ps_e2,
            tc.tile_pool(name="ps_g", bufs=2, space="PSUM") as ps_g,
        ):
            # ---- constants ----
            wqk_sb = singles.tile([128, CC, 128], F16)
            nc.sync.dma_start(out=wqk_sb, in_=wqk.ap())
            bias2_sb = singles.tile([128, 128], F16)
            nc.sync.dma_start(out=bias2_sb, in_=bias2.ap())
            ones2_sb = singles.tile([128, 512], F16)
            nc.sync.dma_start(out=ones2_sb, in_=ones2.ap())
            wv4_sb = singles.tile([128, CC, CC, 128], BF16)
            nc.sync.dma_start(out=wv4_sb, in_=wv4d.ap())
            gbv_sb = singles.tile([128, CC], F32)
            nc.sync.dma_start(out=gbv_sb, in_=gbvd.ap())
            ident = singles.tile([W, W], F32)
            make_identity(nc, ident)
            ones1 = singles.tile([1, 128], BF16)
            nc.vector.memset(ones1, 1.0)
            onesc = singles.tile([128, 1], BF16)
            nc.vector.memset(onesc, 1.0)

            q_sb = singles.tile([128, NPIX], F16)
            k_sb = singles.tile([128, NPIX], F16)
            nc.vector.memset(q_sb[CQ:128, :], 0.0)
            nc.vector.memset(k_sb[CQ:128, :], 0.0)
            z_sb = singles.tile([128, CC, PX], BF16)
            scol_flat = singles.tile([1, W * R], BF16)  # x-major sums
            srow_flat = singles.tile([1, PX], BF16)     # j-major sums
            scolg = singles.tile([W, R], F32)
            srowg = singles.tile([R, W], F32)
            stot = singles.tile([R, W], F32)
            recs = singles.tile([R, W], F32)
            recflat = singles.tile([1, PX], BF16)

            q3 = q_sb.rearrange("p (y x) -> p y x", x=W)
            k3 = k_sb.rearrange("p (y x) -> p y x", x=W)

            # ---- projections: qk = [wq|wk] @ x + bqk (fp16) ----
            for p0, n in _groups(NPIX, SLAB):
                xc_t = xcp.tile([128, SLAB, CC], F16)
                nc.gpsimd.dma_start(out=xc_t[:, :n, :], in_=xcd.ap()[:, p0:p0 + n, :])
                for q0, m in _groups(n, 512):
                    ps = ps_g.tile([128, 512], F32, tag="g")
                    for cc in range(CC):
                        nc.tensor.matmul(ps[:, :m], wqk_sb[:, cc, :],
                                         xc_t[:, q0:q0 + m, cc],
                                         start=(cc == 0), stop=False)
                    nc.tensor.matmul(ps[:, :m], bias2_sb, ones2_sb[:, :m],
                                     start=False, stop=True)
                    nc.scalar.activation(q_sb[0:CQ, p0 + q0:p0 + q0 + m],
                                         ps[0:CQ, :m], AF.Copy)
                    kst = kstp.tile([128, 512], F16, tag="kst")
                    nc.scalar.activation(kst[CQ:128, :m], ps[CQ:128, :m], AF.Copy)
                    # mirror k (partitions 64-127) down to partitions 0-63 via
                    # the scalar HWDGE ring
                    nc.scalar.dma_start(out=k_sb[0:CQ, p0 + q0:p0 + q0 + m],
                                        in_=kst[CQ:128, :m])

            # ---- column phase ----
            for x0, gd in _groups(W, GDC):
                xtc_t = xtcp.tile([128, GDC, C], BF16)
                nc.gpsimd.dma_start(out=xtc_t[:, :gd, :], in_=xtcd.ap()[:, x0:x0 + gd, :])
                for s0, g in _groups(gd, GP):
                    x1 = x0 + s0
                    psT = ps_e2.tile([W, GP, R], F32, tag="e2")
                    for gi in range(g):
                        x = x1 + gi
                        q_col = q3[:, 0:R, x]
                        k_col = k3[:, :, x]
                        nc.tensor.matmul(psT[:, gi, :], k_col, q_col)
                    pt = ptp.tile([128, GP, R], BF16, tag="pt")
                    nc.vector.memset(pt[96:128, :, :], 0.0)
                    nc.scalar.activation(pt[0:W, :g, :], psT[:, :g, :], AF.Exp)
                    nc.gpsimd.affine_select(
                        pt[0:W, :g, :], pt[0:W, :g, :],
                        pattern=[[0, g], [-1, R]], compare_op=ALU.not_equal,
                        fill=0.0, base=0, channel_multiplier=1)
                    psS = ps_e2.tile([1, GP * R], F32, tag="e2")
                    nc.tensor.matmul(psS[:, :g * R], onesc,
                                     pt.rearrange("p g r -> p (g r)")[:, :g * R])
                    nc.vector.tensor_copy(
                        scol_flat[0:1, x1 * R:(x1 + g) * R], psS[:, :g * R])
                    psG = ps_g.tile([128, CC, GP, R], F32, tag="g")
                    for gi in range(g):
                        x = x1 + gi
                        for cc in range(CC):
                            nc.tensor.matmul(
                                psG[:, cc, gi, :],
                                xtc_t[:, s0 + gi, cc * 128:(cc + 1) * 128],
                                pt[:, gi, :])
                        zv = z_sb.rearrange("p c (y x) -> p c y x", x=W)[:, :, :, x]
                        if x % 2 == 0:
                            nc.scalar.activation(zv, psG[:, :, gi, :], AF.Copy)
                        else:
                            nc.vector.tensor_copy(zv, psG[:, :, gi, :])

            # ---- row phase ----
            for j0, gd in _groups(R, GDR):
                xtr_t = xtrp.tile([128, GDR, C], BF16)
                nc.gpsimd.dma_start(out=xtr_t[:, :gd, :], in_=xtrd.ap()[:, j0:j0 + gd, :])
                for s0, g in _groups(gd, GP):
                    j1 = j0 + s0
                    psT = ps_e2.tile([W, GP, W], F32, tag="e2")
                    for gi in range(g):
                        j = j1 + gi
                        q_row = q_sb[:, j * W:(j + 1) * W]
                        k_row = k_sb[:, j * W:(j + 1) * W]
                        nc.tensor.matmul(psT[:, gi, :], k_row, q_row)
                    pt = ptp.tile([128, GP, W], BF16, tag="pt")
                    nc.vector.memset(pt[96:128, :, :], 0.0)
                    nc.scalar.activation(pt[0:W, :g, :], psT[:, :g, :], AF.Exp)
                    psS = ps_e2.tile([1, GP * W], F32, tag="e2")
                    nc.tensor.matmul(psS[:, :g * W], onesc,
                                     pt.rearrange("p g r -> p (g r)")[:, :g * W])
                    nc.vector.tensor_copy(
                        srow_flat[0:1, j1 * W:(j1 + g) * W], psS[:, :g * W])
                    for gi in range(g):
                        j = j1 + gi
                        psG = ps_g.tile([128, CC, W], F32, tag="g")
                        for cc in range(CC):
                            nc.tensor.matmul(
                                psG[:, cc, :],
                                xtr_t[:, s0 + gi, cc * 128:(cc + 1) * 128],
                                pt[:, gi, :])
                        nc.vector.tensor_add(z_sb[:, :, j * W:(j + 1) * W], psG,
                                             z_sb[:, :, j * W:(j + 1) * W])

            # ---- S merge: stot[j, x] = scol^T + srow; recs = gamma / stot ----
            nc.gpsimd.dma_start(out=scolg[:, :],
                                in_=scol_flat.rearrange("p (x j) -> p x j", j=R))
            nc.gpsimd.dma_start(out=srowg[:, :],
                                in_=srow_flat.rearrange("p (j x) -> p j x", x=W))
            psS2 = ps_e2.tile([R, W], F32, tag="e2")
            nc.tensor.transpose(psS2, scolg[:, :], ident[:, :])
            nc.vector.tensor_add(stot, srowg, psS2)
            nc.vector.reciprocal(recs, stot)
            nc.vector.tensor_scalar_mul(recs, recs, gamma)
            # flatten [49, 97] grid to a [1, PX] pixel vector
            nc.gpsimd.dma_start(out=recflat.rearrange("p (j x) -> p j x", x=W),
                                in_=recs[:, :])
            # ---- phase 4: out = (Wv@U + bv x S) * (gamma/S) + x ----
            for t0, tn in _groups(PX, T4):
                psB = ps_e2.tile([128, T4], F32, tag="e2")
                nc.tensor.matmul(psB[:, :tn], ones1[:, :], recflat[:, t0:t0 + tn])
                sbt = sbtp.tile([128, T4], BF16)
                nc.vector.tensor_copy(sbt[:, :tn], psB[:, :tn])
                xr_t = xresp.tile([128, CC, T4], F16)
                nc.gpsimd.dma_start(out=xr_t[:, :, :tn], in_=xresd.ap()[:, :, t0:t0 + tn])
                outst = outp.tile([128, CC, T4], F32)
                for cco in range(CC):
                    psO = ps_g.tile([128, T4], F32, tag="g")
                    for cci in range(CC):
                        nc.tensor.matmul(psO[:, :tn], wv4_sb[:, cci, cco, :],
                                         z_sb[:, cci, t0:t0 + tn],
                                         start=(cci == 0), stop=(cci == CC - 1))
                    nc.vector.tensor_mul(outst[:, cco, :tn], psO[:, :tn],
                                         sbt[:, :tn])
                    # out = (psO*gamma/S + gamma*bv) + x
                    nc.vector.scalar_tensor_tensor(
                        outst[:, cco, :tn], outst[:, cco, :tn],
                        gbv_sb[:, cco:cco + 1], xr_t[:, cco, :tn],
                        op0=ALU.add, op1=ALU.add)
                nc.gpsimd.dma_start(out=out.ap()[:, :, t0:t0 + tn],
                                    in_=outst[:, :, :tn])

    nc.compile()
    return nc


def _prep_core(x, n, half):
    y0 = half * 48
    xs = np.roll(x[n], -y0, axis=1)  # [C, H, W] fp32
    xcd_h = np.ascontiguousarray(
        xs.reshape(CC, 128, NPIX).transpose(1, 2, 0)).astype(np.float16)
    # padded to 128 partitions (rows 97-127 zero) so agg stationaries hit FWL
    xtcd_h = np.zeros((128, W, C), ml_dtypes.bfloat16)
    xtcd_h[:W] = xs.transpose(1, 2, 0).astype(ml_dtypes.bfloat16)
    xtrd_h = np.zeros((128, R, C), ml_dtypes.bfloat16)
    xtrd_h[:W] = xs[:, :R, :].transpose(2, 1, 0).astype(ml_dtypes.bfloat16)
    xresd_h = np.ascontiguousarray(
        xs[:, :R, :].reshape(CC, 128, PX).transpose(1, 0, 2)).astype(np.float16)
    return {"xcd": xcd_h, "xtcd": xtcd_h, "xtrd": xtrd_h, "xresd": xresd_h}


def kernel(x, wq, bq, wk, bk, wv, bv, gamma):
    global last_results
    x = np.asarray(x, dtype=np.float32)
    gamma_f = float(np.asarray(gamma).reshape(-1)[0])

    if "nc" not in _cache:
        _cache["nc"] = _build(gamma_f)
    nc = _cache["nc"]

    wqk_h = np.ascontiguousarray(
        np.concatenate([np.asarray(wq).T, np.asarray(wk).T], axis=1)
        .reshape(CC, 128, 128).transpose(1, 0, 2)).astype(np.float16)
    bias2_h = np.zeros((128, 128), np.float16)
    bias2_h[0] = np.concatenate([np.asarray(bq), np.asarray(bk)]).astype(np.float16)
    ones2_h = np.ones((128, 512), np.float16)
    wv4_h = np.ascontiguousarray(
        np.asarray(wv).T.reshape(CC, 128, CC, 128).transpose(1, 0, 2, 3)
    ).astype(ml_dtypes.bfloat16)
    gbv_h = np.ascontiguousarray(
        (gamma_f * np.asarray(bv)).reshape(CC, 128).T).astype(np.float32)

    shared = {"wqk": wqk_h, "bias2": bias2_h, "ones2": ones2_h,
              "wv4d": wv4_h, "gbvd": gbv_h}
    in_maps = []
    for core in range(8):
        m = _prep_core(x, core // 2, core % 2)
        m.update(shared)
        in_maps.append(m)

    last_results = run_bass_kernel_spmd(
        nc, in_maps, core_ids=list(range(8)),
        trace=os.environ.get("KERNEL_TRACE") == "1")

    full = np.empty((4, C, H, W), np.float32)
    for core in range(8):
        n, half = core // 2, core % 2
        y0 = half * 48
        o = last_results.results[core]["out"]  # [128, CC, PX]
        rows = (np.arange(R) + y0) % H
        full[n][:, rows, :] = o.transpose(1, 0, 2).reshape(C, R, W)
    return full


# revision 27
# speedup vs baseline: 1.0188x; 1.0188x over previous
"""CrissCrossAttention Trainium2 kernel (v2).

Sharding: 8 cores = 4 samples x 2 row-halves. Each core's sample is rolled so
its 49 rows sit at local rows [0, 49) (column attention is permutation-
invariant over the column index, so rolling is safe and keeps the SPMD
program identical across cores).

Math (per core, local rows j in [0,49), cols x in [0,97)):
  q|k = Wqk @ x + bqk                  (fp16 matmuls, PSUM fp32)
  E_row[(j,x), i] = q(j,x).k(j,i)      E_col[(j,x), i] = q(j,x).k(i,x)
  col diag (i==j) masked to -300 in PSUM pre-exp; P = exp(E) unnormalized
  (no max-shift; |E| <~ 45 fits fp32); S accumulated by the exp's accum_out.
  U = P_row @ X_row + P_col @ X_col    (bf16, channel-major in z)
  out = (Wv @ U + bv x S) * (gamma/S) + x     (rank-1 bias matmul makes the
  final stage a pure scale-and-add; exact by linearity)

DMA strategy: all bulk transfers ride SWDGE (nc.gpsimd) which sprays
descriptors across all 16 SDMA engines; host-side layouts are chosen so
every transfer has multi-KB contiguous per-partition lines.  The k mirror
(partition shift) uses the scalar HWDGE ring to stay off the sync ring.
"""

import os
import numpy as np
import ml_dtypes

import concourse.bacc as bacc
import concourse.bass as bass
import concourse.tile as tile
import concourse.mybir as mybir
from concourse.bass_utils import run_bass_kernel_spmd
from concourse.masks import make_identity

F32 = mybir.dt.float32
F16 = mybir.dt.float16
BF16 = mybir.dt.bfloat16
AF = mybir.ActivationFunctionType
AX = mybir.AxisListType
ALU = mybir.AluOpType

C = 512
CC = 4          # 4 chunks of 128 channels
CQ = 64
H = W = 97
NPIX = H * W    # 9409
R = 49          # rows per core (halves overlap at row 48)
PX = R * W      # 4753
GP = 4          # softmax/agg group
GDC = 12        # xtc chunk (x's per dma)
GDR = 8         # xtr chunk (j's per dma)
SLAB = 512      # projection slab (pixels per dma)
T4 = 512        # phase-4 pixel tile

_cache = {}
last_results = None


def _groups(total, g):
    out = []
    i = 0
    while i < total:
        out.append((i, min(g, total - i)))
        i += g
    return out


def _build(gamma: float):
    nc = bacc.Bacc("TRN2", target_bir_lowering=False, debug=False,
                   enable_asserts=False)

    xcd = nc.dram_tensor("xcd", [128, NPIX, CC], F16, kind="ExternalInput")
    xtcd = nc.dram_tensor("xtcd", [128, W, C], BF16, kind="ExternalInput")
    xtrd = nc.dram_tensor("xtrd", [128, R, C], BF16, kind="ExternalInput")
    xresd = nc.dram_tensor("xresd", [128, CC, PX], F16, kind="ExternalInput")
    wqk = nc.dram_tensor("wqk", [128, CC, 128], F16, kind="ExternalInput")
    bqvd = nc.dram_tensor("bqvd", [128, 1], F32, kind="ExternalInput")
    wv4d = nc.dram_tensor("wv4d", [128, CC, CC, 128], BF16, kind="ExternalInput")
    gbvd = nc.dram_tensor("gbvd", [128, CC], F32, kind="ExternalInput")
    out = nc.dram_tensor("out", [128, CC, PX], F32, kind="ExternalOutput")
    dbg = None

    with tile.TileContext(nc) as tc:
        with (
            tc.tile_pool(name="singles", bufs=1) as singles,
            tc.tile_pool(name="xcp", bufs=2) as xcp,
            tc.tile_pool(name="kstp", bufs=2) as kstp,
            tc.tile_pool(name="xtcp", bufs=2) as xtcp,
            tc.tile_pool(name="xtrp", bufs=2) as xtrp,
            tc.tile_pool(name="xresp", bufs=2) as xresp,
            tc.tile_pool(name="ptp", bufs=2) as ptp,
            tc.tile_pool(name="outp", bufs=2) as outp,
            tc.tile_pool(name="sbtp", bufs=2) as sbtp,
            tc.tile_pool(name="ps_e2", bufs=2, space="PSUM") as ps_e2,
            tc.tile_pool(name="ps_s", bufs=2, space="PSUM") as ps_s,
            tc.tile_pool(name="ps_g", bufs=2, space="PSUM") as ps_g,
        ):
            # ---- constants ----
            wqk_sb = singles.tile([128, CC, 128], F16)
            nc.sync.dma_start(out=wqk_sb, in_=wqk.ap())
            bqv_sb = singles.tile([128, 1], F32)
            nc.sync.dma_start(out=bqv_sb, in_=bqvd.ap())
            wv4_sb = singles.tile([128, CC, CC, 128], BF16)
            nc.sync.dma_start(out=wv4_sb, in_=wv4d.ap())
            gbv_sb = singles.tile([128, CC], F32)
            nc.sync.dma_start(out=gbv_sb, in_=gbvd.ap())
            ident = singles.tile([W, W], F32)
            make_identity(nc, ident)
            ones1 = singles.tile([1, 128], BF16)
            nc.vector.memset(ones1, 1.0)
            onesc = singles.tile([128, 1], BF16)
            nc.vector.memset(onesc, 1.0)

            q_sb = singles.tile([128, NPIX], F16)
            k_sb = singles.tile([128, NPIX], F16)
            nc.vector.memset(q_sb[CQ:128, :], 0.0)
            nc.vector.memset(k_sb[CQ:128, :], 0.0)
            z_sb = singles.tile([128, CC, PX], BF16)
            scol_flat = singles.tile([1, W * R], BF16)  # x-major sums
            srow_flat = singles.tile([1, PX], BF16)     # j-major sums
            scolg = singles.tile([W, R], F32)
            srowg = singles.tile([R, W], F32)
            stot = singles.tile([R, W], F32)
            recs = singles.tile([R, W], F32)
            recflat = singles.tile([1, PX], BF16)

            # pixel order is x-major: pix = x*H + y
            q3 = q_sb.rearrange("p (x y) -> p x y", y=H)
            k3 = k_sb.rearrange("p (x y) -> p x y", y=H)

            # ---- projections: qk = [wq|wk] @ x + bqk (fp16) ----
            for p0, n in _groups(NPIX, SLAB):
                xc_t = xcp.tile([128, SLAB, CC], F16)
                nc.gpsimd.dma_start(out=xc_t[:, :n, :], in_=xcd.ap()[:, p0:p0 + n, :])
                for q0, m in _groups(n, 512):
                    ps = ps_g.tile([128, 512], F32, tag="g")
                    for cc in range(CC):
                        nc.tensor.matmul(ps[:, :m], wqk_sb[:, cc, :],
                                         xc_t[:, q0:q0 + m, cc],
                                         start=(cc == 0), stop=(cc == CC - 1))
                    nc.vector.tensor_scalar_add(
                        q_sb[0:CQ, p0 + q0:p0 + q0 + m], ps[0:CQ, :m],
                        bqv_sb[0:CQ, :])
                    kst = kstp.tile([128, 512], F16, tag="kst")
                    nc.vector.tensor_scalar_add(
                        kst[CQ:128, :m], ps[CQ:128, :m], bqv_sb[CQ:128, :])
                    # mirror k (partitions 64-127) down to partitions 0-63 via
                    # the scalar HWDGE ring
                    nc.scalar.dma_start(out=k_sb[0:CQ, p0 + q0:p0 + q0 + m],
                                        in_=kst[CQ:128, :m])

            # ---- column phase ----
            for x0, gd in _groups(W, GDC):
                xtc_t = xtcp.tile([128, GDC, C], BF16)
                nc.gpsimd.dma_start(out=xtc_t[:, :gd, :], in_=xtcd.ap()[:, x0:x0 + gd, :])
                for s0, g in _groups(gd, GP):
                    x1 = x0 + s0
                    psT = ps_e2.tile([W, GP, R], F32, tag="e2")
                    for gi in range(g):
                        x = x1 + gi
                        q_col = q3[:, x, 0:R]
                        k_col = k3[:, x, :]
                        nc.tensor.matmul(psT[:, gi, :], k_col, q_col)
                    pt = ptp.tile([128, GP, R], BF16, tag="pt")
                    nc.vector.memset(pt[96:128, :, :], 0.0)
                    nc.scalar.activation(pt[0:W, :g, :], psT[:, :g, :], AF.Exp)
                    nc.gpsimd.affine_select(
                        pt[0:W, :g, :], pt[0:W, :g, :],
                        pattern=[[0, g], [-1, R]], compare_op=ALU.not_equal,
                        fill=0.0, base=0, channel_multiplier=1)
                    psS = ps_s.tile([1, GP * R], F32, tag="s")
                    nc.tensor.matmul(psS[:, :g * R], onesc,
                                     pt.rearrange("p g r -> p (g r)")[:, :g * R])
                    nc.vector.tensor_copy(
                        scol_flat[0:1, x1 * R:(x1 + g) * R], psS[:, :g * R])
                    psG = ps_g.tile([128, CC, GP, R], F32, tag="g")
                    for gi in range(g):
                        x = x1 + gi
                        for cc in range(CC):
                            nc.tensor.matmul(
                                psG[:, cc, gi, :],
                                xtc_t[:, s0 + gi, cc * 128:(cc + 1) * 128],
                                pt[:, gi, :])
                        zv = z_sb.rearrange("p c (y x) -> p c y x", x=W)[:, :, :, x]
                        if x % 2 == 0:
                            nc.scalar.activation(zv, psG[:, :, gi, :], AF.Copy)
                        else:
                            nc.vector.tensor_copy(zv, psG[:, :, gi, :])

            # ---- row phase ----
            for j0, gd in _groups(R, GDR):
                xtr_t = xtrp.tile([128, GDR, C], BF16)
                nc.gpsimd.dma_start(out=xtr_t[:, :gd, :], in_=xtrd.ap()[:, j0:j0 + gd, :])
                for s0, g in _groups(gd, GP):
                    j1 = j0 + s0
                    psT = ps_e2.tile([W, GP, W], F32, tag="e2")
                    for gi in range(g):
                        j = j1 + gi
                        q_row = q3[:, :, j]
                        k_row = k3[:, :, j]
                        nc.tensor.matmul(psT[:, gi, :], k_row, q_row)
                    pt = ptp.tile([128, GP, W], BF16, tag="pt")
                    nc.vector.memset(pt[96:128, :, :], 0.0)
                    nc.scalar.activation(pt[0:W, :g, :], psT[:, :g, :], AF.Exp)
                    psS = ps_s.tile([1, GP * W], F32, tag="s")
                    nc.tensor.matmul(psS[:, :g * W], onesc,
                                     pt.rearrange("p g r -> p (g r)")[:, :g * W])
                    nc.vector.tensor_copy(
                        srow_flat[0:1, j1 * W:(j1 + g) * W], psS[:, :g * W])
                    for gi in range(g):
                        j = j1 + gi
                        psG = ps_g.tile([128, CC, W], F32, tag="g")
                        for cc in range(CC):
                            nc.tensor.matmul(
                                psG[:, cc, :],
                                xtr_t[:, s0 + gi, cc * 128:(cc + 1) * 128],
                                pt[:, gi, :])
                        nc.vector.tensor_add(z_sb[:, :, j * W:(j + 1) * W], psG,
                                             z_sb[:, :, j * W:(j + 1) * W])

            # ---- S merge: stot[j, x] = scol^T + srow; recs = gamma / stot ----
            nc.gpsimd.dma_start(out=scolg[:, :],
                                in_=scol_flat.rearrange("p (x j) -> p x j", j=R))
            nc.gpsimd.dma_start(out=srowg[:, :],
                                in_=srow_flat.rearrange("p (j x) -> p j x", x=W))
            psS2 = ps_s.tile([R, W], F32, tag="s")
            nc.tensor.transpose(psS2, scolg[:, :], ident[:, :])
            nc.vector.tensor_add(stot, srowg, psS2)
            nc.vector.reciprocal(recs, stot)
            nc.vector.tensor_scalar_mul(recs, recs, gamma)
            # flatten [49, 97] grid to a [1, PX] pixel vector
            nc.gpsimd.dma_start(out=recflat.rearrange("p (j x) -> p j x", x=W),
                                in_=recs[:, :])
            # ---- phase 4: out = (Wv@U + bv x S) * (gamma/S) + x ----
            for t0, tn in _groups(PX, T4):
                psB = ps_s.tile([128, T4], F32, tag="s")
                nc.tensor.matmul(psB[:, :tn], ones1[:, :], recflat[:, t0:t0 + tn])
                sbt = sbtp.tile([128, T4], BF16)
                nc.vector.tensor_copy(sbt[:, :tn], psB[:, :tn])
                xr_t = xresp.tile([128, CC, T4], F16)
                nc.gpsimd.dma_start(out=xr_t[:, :, :tn], in_=xresd.ap()[:, :, t0:t0 + tn])
                outst = outp.tile([128, CC, T4], F32)
                for cco in range(CC):
                    psO = ps_g.tile([128, T4], F32, tag="g")
                    for cci in range(CC):
                        nc.tensor.matmul(psO[:, :tn], wv4_sb[:, cci, cco, :],
                                         z_sb[:, cci, t0:t0 + tn],
                                         start=(cci == 0), stop=(cci == CC - 1))
                    nc.vector.tensor_mul(outst[:, cco, :tn], psO[:, :tn],
                                         sbt[:, :tn])
                    # out = (psO*gamma/S + gamma*bv) + x
                    nc.vector.scalar_tensor_tensor(
                        outst[:, cco, :tn], outst[:, cco, :tn],
                        gbv_sb[:, cco:cco + 1], xr_t[:, cco, :tn],
                        op0=ALU.add, op1=ALU.add)
                nc.gpsimd.dma_start(out=out.ap()[:, :, t0:t0 + tn],
                                    in_=outst[:, :, :tn])

    nc.compile()
    return nc


def _prep_core(x, n, half):
    y0 = half * 48
    xs = np.roll(x[n], -y0, axis=1)  # [C, H, W] fp32
    xcd_h = np.ascontiguousarray(
        xs.transpose(0, 2, 1).reshape(CC, 128, NPIX).transpose(1, 2, 0)
    ).astype(np.float16)
    # padded to 128 partitions (rows 97-127 zero) so agg stationaries hit FWL
    xtcd_h = np.zeros((128, W, C), ml_dtypes.bfloat16)
    xtcd_h[:W] = xs.transpose(1, 2, 0).astype(ml_dtypes.bfloat16)
    xtrd_h = np.zeros((128, R, C), ml_dtypes.bfloat16)
    xtrd_h[:W] = xs[:, :R, :].transpose(2, 1, 0).astype(ml_dtypes.bfloat16)
    xresd_h = np.ascontiguousarray(
        xs[:, :R, :].reshape(CC, 128, PX).transpose(1, 0, 2)).astype(np.float16)
    return {"xcd": xcd_h, "xtcd": xtcd_h, "xtrd": xtrd_h, "xresd": xresd_h}


def kernel(x, wq, bq, wk, bk, wv, bv, gamma):
    global last_results
    x = np.asarray(x, dtype=np.float32)
    gamma_f = float(np.asarray(gamma).reshape(-1)[0])

    if "nc" not in _cache:
        _cache["nc"] = _build(gamma_f)
    nc = _cache["nc"]

    wqk_h = np.ascontiguousarray(
        np.concatenate([np.asarray(wq).T, np.asarray(wk).T], axis=1)
        .reshape(CC, 128, 128).transpose(1, 0, 2)).astype(np.float16)
    bqv_h = np.concatenate([np.asarray(bq), np.asarray(bk)])[:, None].astype(np.float32)
    wv4_h = np.ascontiguousarray(
        np.asarray(wv).T.reshape(CC, 128, CC, 128).transpose(1, 0, 2, 3)
    ).astype(ml_dtypes.bfloat16)
    gbv_h = np.ascontiguousarray(
        (gamma_f * np.asarray(bv)).reshape(CC, 128).T).astype(np.float32)

    shared = {"wqk": wqk_h, "bqvd": bqv_h, "wv4d": wv4_h, "gbvd": gbv_h}
    in_maps = []
    for core in range(8):
        m = _prep_core(x, core // 2, core % 2)
        m.update(shared)
        in_maps.append(m)

    last_results = run_bass_kernel_spmd(
        nc, in_maps, core_ids=list(range(8)),
        trace=os.environ.get("KERNEL_TRACE") == "1")

    full = np.empty((4, C, H, W), np.float32)
    for core in range(8):
        n, half = core // 2, core % 2
        y0 = half * 48
        o = last_results.results[core]["out"]  # [128, CC, PX]
        rows = (np.arange(R) + y0) % H
        full[n][:, rows, :] = o.transpose(1, 0, 2).reshape(C, R, W)
    return full
